# revision 23
# baseline (speedup 1.0000x reference)
"""BayesianKAN ECG kernel v2 for Trainium2 (8 NeuronCores, data-parallel).

Per-core pipeline over quads of 2048 rows (4 quads/core):
  x (bf16, host k-major shuffled) --DMA--> [128,4,1000] tiles
  --gp add + DVE add-tree--> pooled[128,16,100] fp32
  --reduce/ttr stats + Newton rsqrt--> mean/rstd --ts apply--> xn
  --PE transpose--> xnt[100,2048] fp32 (via DMA PSUM->SBUF)
  L1 basis (M1=11 compressed RBF, anchors m=0,6):
    ACT: Square+Exp anchors, Exp ratios R1=e^{2AD x}, R2=e^{4AD x} (bf16)
    DVE/GP: R4=R2*R2 and 9 slice products G_m = G_src * R^pw
  mm1: 22 bf16 matmuls -> h[64,2048] -> Tanh -> hb
  L2: transpose roundtrip for norm stats, apply -> back-transpose
    -> replicate DMA -> b2s[128,2048] fp32
    basis (M2=12, 6 chunks x 2 halves, anchor chunks 0,3): ACT Square/Exp +
    4 chunk products; mm2: 12 bf16 matmuls -> out[5,2048] -> DMA out.
Numerics: basis compression via host least-squares projection (16->M RBFs),
ratio-recurrence constants folded into w1/w2 host-side. Expected rel err ~1.2e-2
(validated by sim_v2.py host simulation).
"""

import os
import sys
from contextlib import ExitStack

import numpy as np
import ml_dtypes

sys.path.insert(0, "/opt/trn_rl_repo")

import concourse.bass as bass
import concourse.tile as tile
from concourse import mybir
from concourse.bass_utils import run_bass_kernel_spmd

from concourse import bass2jax as _b2j

_orig_hook = _b2j.neuronx_cc_hook


def _dbg_hook(*a, **k):
    try:
        return _orig_hook(*a, **k)
    except BaseException:
        import traceback
        with open("/tmp/hook_err.txt", "w") as f:
            traceback.print_exc(file=f)
        raise


_b2j.neuronx_cc_hook = _dbg_hook

MAX_WAITS = 1


def _split_sync_waits(nc, limit=MAX_WAITS):
    """Walrus in this env rejects instructions with more than ~2 sync waits.

    Move excess waits onto same-engine NOPs inserted right before the
    offending instruction (in-order engines make this equivalent).
    """
    n_split = 0
    for block in nc.main_func.blocks:
        new_insts = []
        for inst in block.instructions:
            si = inst.sync_info
            waits = list(si.on_wait) if si is not None else []
            if len(waits) > limit:
                extra, keep = waits[:-limit], waits[-limit:]
                for k in range(0, len(extra), limit):
                    nop = mybir.InstNoOp(
                        name=f"{inst.name}-ws{k}",
                        sync_info=mybir.SyncInfo(
                            on_wait=extra[k : k + limit], on_update=[]
                        ),
                        bass_nofuse=True,
                        engine=inst.engine,
                    )
                    nc.register_instruction(nop, overwrite=True)
                    new_insts.append(nop)
                    n_split += 1
                si.on_wait = keep
                inst.sync_info = si
            new_insts.append(inst)
        block.instructions[:] = new_insts
    return n_split


BATCH = 65536
SEQ = 1000
IN_DIM = 100
POOLW = 10
HID = 64
OUT_DIM = 5
NB = 16
NCORES = 8
ROWS = BATCH // NCORES          # 8192 rows per core
TILE_P = 128
PBLK = 1024                     # rows per quad
NSUBQ = PBLK // TILE_P          # subtiles per quad
NBLK2 = NSUBQ // 2              # dma blocks per quad (2 subtiles each)
NCH = PBLK // 512               # 512-col chunks per quad

M1 = 11
M2 = 12
H2C = M2 // 2                   # 6 chunks
RBF_A = float(0.5 / 0.36)

# L1 slice plan: m -> (src_m, ratio_power)
ANC1 = (0, 6)
PLAN1 = {1: (0, 1), 2: (0, 2), 3: (1, 2), 4: (0, 4), 5: (1, 4),
         7: (6, 1), 8: (6, 2), 9: (7, 2), 10: (6, 4)}
# L2 chunk plan: c -> (src_c, ratio_power)
ANC2 = (0, 3)
PLAN2 = {1: (0, 1), 2: (0, 2), 4: (3, 1), 5: (3, 2)}
MM1_ORDER = [0, 6, 1, 7, 2, 8, 3, 9, 4, 5, 10]
MM2_ORDER = [0, 3, 1, 4, 2, 5]
# engine routing for slice/chunk products ('gp' or 'dve')
ENG1 = {m: 'dve' for m in PLAN1}
ENG2 = {c: 'dve' for c in PLAN2}

LAST_RESULTS = None

F32 = mybir.dt.float32
BF16 = mybir.dt.bfloat16
I32 = mybir.dt.int32
AF = mybir.ActivationFunctionType
ALU = mybir.AluOpType
AX = mybir.AxisListType
MAGIC = 0x5F3759DF


def _ensure_ntff_hook():
    """Synthesize antenv.axon_hooks (absent in this image) so trace=True works."""
    import types

    if "antenv.axon_hooks" in sys.modules:
        return
    mod = types.ModuleType("antenv.axon_hooks")
    mod._hook = None

    def set_axon_ntff_profile_hook(h):
        mod._hook = h

    def get_axon_ntff_profile_hook():
        return mod._hook

    mod.set_axon_ntff_profile_hook = set_axon_ntff_profile_hook
    mod.get_axon_ntff_profile_hook = get_axon_ntff_profile_hook
    sys.modules["antenv.axon_hooks"] = mod
    import antenv

    antenv.axon_hooks = mod
    try:
        from trn_agent_boot.trn_boot import _ntff_profile_via_ctypes

        hook = _ntff_profile_via_ctypes("/opt/axon/libaxon_pjrt.so")
        if hook is not None:
            set_axon_ntff_profile_hook(hook)
    except Exception as e:
        print("ntff hook setup failed:", e)


def _newton_rsqrt(nc, pool, var_ap, out_ap, n_par, n_free, scale):
    """out = rsqrt(var * scale) elementwise; Newton w/ bit-trick seed."""
    v = pool.tile([n_par, n_free], F32, tag="nw_v")
    y = pool.tile([n_par, n_free], F32, tag="nw_y")
    t = pool.tile([n_par, n_free], F32, tag="nw_t")
    nc.vector.tensor_scalar(v, var_ap, float(scale), None, ALU.mult)
    nc.vector.tensor_scalar(
        y.bitcast(I32), v.bitcast(I32), 1, None, ALU.logical_shift_right
    )
    nc.vector.tensor_scalar(
        y.bitcast(I32), y.bitcast(I32), -1, MAGIC, ALU.mult, ALU.add
    )
    for it in range(2):
        nc.vector.tensor_mul(t, y, y)
        nc.vector.tensor_mul(t, t, v)
        nc.vector.tensor_scalar(t, t, -0.5, 1.5, ALU.mult, ALU.add)
        nc.vector.tensor_mul(out_ap if it == 1 else y, y, t)


def build_bass(centers_np, rows=ROWS):
    nq = rows // PBLK
    assert rows % PBLK == 0
    nc = bass.Bass()

    c16 = np.asarray(centers_np, np.float64)
    cM1 = np.linspace(c16[0], c16[-1], M1)
    cM2 = np.linspace(c16[0], c16[-1], M2)
    D1 = float(cM1[1] - cM1[0])
    D2 = float(cM2[1] - cM2[0])
    A = RBF_A

    x_in = nc.declare_dram_parameter("x", [rows, SEQ], BF16, isOutput=False)
    w1_in = nc.declare_dram_parameter("w1", [IN_DIM, M1 * HID], BF16, isOutput=False)
    w2_in = nc.declare_dram_parameter("w2", [TILE_P, H2C * OUT_DIM], BF16, isOutput=False)
    cb1a_in = nc.declare_dram_parameter("cb1a", [IN_DIM, 2], F32, isOutput=False)
    cb2a_in = nc.declare_dram_parameter("cb2a", [TILE_P, 2], F32, isOutput=False)
    ident_in = nc.declare_dram_parameter("ident", [TILE_P, TILE_P], F32, isOutput=False)
    out_ext = nc.declare_dram_parameter("out", [OUT_DIM, rows], F32, isOutput=True)

    with ExitStack() as ctx:
        tc = ctx.enter_context(tile.TileContext(nc))
        singles = ctx.enter_context(tc.tile_pool(name="singles", bufs=1))
        xin_p = ctx.enter_context(tc.tile_pool(name="xin", bufs=2))
        y5_p = ctx.enter_context(tc.tile_pool(name="y5", bufs=1))
        y2_p = ctx.enter_context(tc.tile_pool(name="y2", bufs=1))
        pool_p = ctx.enter_context(tc.tile_pool(name="pooled", bufs=1))
        sm_p = ctx.enter_context(tc.tile_pool(name="sm", bufs=2))
        nw_p = ctx.enter_context(tc.tile_pool(name="newton", bufs=2))
        xn_p = ctx.enter_context(tc.tile_pool(name="xn", bufs=1))
        xnt_p = ctx.enter_context(tc.tile_pool(name="xnt", bufs=2))
        sq_p = ctx.enter_context(tc.tile_pool(name="sq", bufs=1))
        b1_p = ctx.enter_context(tc.tile_pool(name="b1", bufs=1))
        r1_p = ctx.enter_context(tc.tile_pool(name="r1", bufs=1))
        hb_p = ctx.enter_context(tc.tile_pool(name="hb", bufs=1))
        xn2_p = ctx.enter_context(tc.tile_pool(name="xn2", bufs=2))
        b2s_p = ctx.enter_context(tc.tile_pool(name="b2s", bufs=1))
        rb_p = ctx.enter_context(tc.tile_pool(name="rb", bufs=1))
        b2_p = ctx.enter_context(tc.tile_pool(name="b2", bufs=1))
        outs_p = ctx.enter_context(tc.tile_pool(name="outs", bufs=2))
        ps_xnt = ctx.enter_context(tc.tile_pool(name="ps_xnt", bufs=2, space="PSUM"))
        ps_h = ctx.enter_context(tc.tile_pool(name="ps_h", bufs=2, space="PSUM"))
        ps_ha = ctx.enter_context(tc.tile_pool(name="ps_ha", bufs=1, space="PSUM"))
        ps_b2t = ctx.enter_context(tc.tile_pool(name="ps_b2t", bufs=1, space="PSUM"))
        ps_o = ctx.enter_context(tc.tile_pool(name="ps_o", bufs=1, space="PSUM"))

        ident = singles.tile([TILE_P, TILE_P], F32)
        nc.sync.dma_start(out=ident, in_=ident_in[:, :])
        w1 = singles.tile([IN_DIM, M1, HID], BF16)
        nc.sync.dma_start(out=w1, in_=w1_in[:, :].rearrange("i (m o) -> i m o", m=M1))
        w2 = singles.tile([TILE_P, H2C, OUT_DIM], BF16)
        nc.sync.dma_start(out=w2, in_=w2_in[:, :].rearrange("p (c o) -> p c o", c=H2C))
        cb1a = singles.tile([IN_DIM, 2], F32)
        nc.sync.dma_start(out=cb1a, in_=cb1a_in[:, :])
        cb2a = singles.tile([TILE_P, 2], F32)
        nc.sync.dma_start(out=cb2a, in_=cb2a_in[:, :])

        # x view: [nq, blk, sub, p, f]
        x_t = x_in[:, :].rearrange(
            "(q b s p) f -> q b s p f", q=nq, b=NBLK2, s=2
        )

        def stage_x(q, y5):
            """Per-quad input DMA + first pooling add on gpsimd."""
            for b in range(NBLK2):
                xt = xin_p.tile([TILE_P, 2, SEQ], BF16, tag="x")
                nc.sync.dma_start(
                    out=xt, in_=x_t[q, b].rearrange("s p f -> p s f")
                )
                nc.gpsimd.tensor_tensor(
                    y5[:, 2 * b : 2 * b + 2, :],
                    xt[:, :, 0:500],
                    xt[:, :, 500:1000],
                    ALU.add,
                )

        def stage_rest(q, y5):
            """Pool tree + stats + normalize + transpose -> xnt SBUF."""
            y2 = y2_p.tile([TILE_P, NSUBQ, 200], BF16, tag="y2")
            nc.vector.tensor_tensor(
                y2, y5[:, :, 0:200], y5[:, :, 200:400], ALU.add
            )
            pooled = pool_p.tile([TILE_P, NSUBQ, IN_DIM], F32, tag="pooled")
            nc.vector.tensor_tensor(
                pooled, y2[:, :, 0:100], y2[:, :, 100:200], ALU.add
            )
            nc.vector.tensor_tensor(
                pooled, pooled, y5[:, :, 400:500], ALU.add
            )
            mu_s = sm_p.tile([TILE_P, NSUBQ], F32, tag="mu_s")
            nc.vector.tensor_reduce(mu_s, pooled, AX.X, ALU.add)
            scr_t = y2_p.tile([TILE_P, NSUBQ, 200], BF16, tag="y2")
            scr = scr_t[:, :, 0:IN_DIM]
            nc.scalar.activation(scr, pooled, AF.Square)
            sq_s = sm_p.tile([TILE_P, NSUBQ], F32, tag="sq_s")
            nc.vector.tensor_reduce(sq_s, scr, AX.X, ALU.add)
            musq = sm_p.tile([TILE_P, NSUBQ], F32, tag="musq")
            nc.vector.tensor_mul(musq, mu_s, mu_s)
            v = sm_p.tile([TILE_P, NSUBQ], F32, tag="v")
            nc.vector.scalar_tensor_tensor(
                v, musq, -1.0 / IN_DIM, sq_s, ALU.mult, ALU.add
            )
            r3 = sm_p.tile([TILE_P, NSUBQ, 1], F32, tag="r3")
            _newton_rsqrt(nc, nw_p, v, r3[:, :, 0], TILE_P, NSUBQ,
                          1.0 / (IN_DIM - 1))
            mur = sm_p.tile([TILE_P, NSUBQ, 1], F32, tag="mur")
            nc.vector.tensor_scalar(
                mur[:, :, 0], mu_s, -1.0 / IN_DIM, None, ALU.mult
            )
            nc.vector.tensor_mul(mur, mur, r3)
            xn_all = xn_p.tile([TILE_P, NSUBQ, IN_DIM], F32, tag="xn_all")
            for s in range(NSUBQ):
                nc.scalar.activation(
                    xn_all[:, s, :], pooled[:, s, :], AF.Identity,
                    bias=mur[:, s, :], scale=r3[:, s, :],
                )
            xnt = xnt_p.tile([IN_DIM, PBLK], F32, tag="xnt")
            for hq in range(NCH):
                xnt_ps = ps_xnt.tile([IN_DIM, 512], F32, tag="xnt_ps")
                for j in range(4):
                    s = 4 * hq + j
                    nc.tensor.transpose(
                        xnt_ps[:, j * TILE_P : (j + 1) * TILE_P],
                        xn_all[:, s, :], ident,
                    )
                nc.scalar.copy(xnt[:, hq * 512 : (hq + 1) * 512], xnt_ps)
            return xnt

        def l1(q, xnt):
            b1 = b1_p.tile([TILE_P, M1, PBLK], BF16, tag="b1")
            r1b = r1_p.tile([TILE_P, PBLK], BF16, tag="r1b")
            r2b = r1_p.tile([TILE_P, PBLK], BF16, tag="r2b")
            r4b = r1_p.tile([TILE_P, PBLK], BF16, tag="r4b")
            for ai, a in enumerate(ANC1):
                sqt = sq_p.tile([IN_DIM, PBLK], F32, tag="sq1")
                nc.scalar.activation(sqt, xnt, AF.Square, bias=cb1a[:, ai : ai + 1])
                nc.scalar.activation(
                    b1[0:IN_DIM, a, :], sqt, AF.Exp, scale=float(-A)
                )
            nc.scalar.activation(
                r1b[0:IN_DIM, :], xnt, AF.Exp, scale=float(2 * A * D1)
            )
            nc.scalar.activation(
                r2b[0:IN_DIM, :], xnt, AF.Exp, scale=float(4 * A * D1)
            )
            # pad rows hold zeros (memset once would cost more than it saves;
            # full-128 ops on them keep the DVE 2x packed mode engaged)
            nc.vector.tensor_mul(r4b, r2b, r2b)
            rpow = {1: r1b, 2: r2b, 4: r4b}
            # emit in dependency order
            for m in [1, 7, 2, 8, 3, 4, 9, 5, 10]:
                src, pw = PLAN1[m]
                eng = nc.gpsimd if ENG1[m] == 'gp' else nc.vector
                eng.tensor_tensor(b1[:, m, :], b1[:, src, :], rpow[pw], ALU.mult)
            hb = hb_p.tile([HID, PBLK], F32, tag="hb")
            for g in range(NCH):
                hps = ps_h.tile([HID, 512], F32, tag="hps")
                for idx, m in enumerate(MM1_ORDER):
                    nc.tensor.matmul(
                        hps,
                        w1[:, m, :],
                        b1[0:IN_DIM, m, g * 512 : (g + 1) * 512],
                        start=(idx == 0),
                        stop=(idx == M1 - 1),
                    )
                nc.scalar.activation(
                    hb[:, g * 512 : (g + 1) * 512], hps, AF.Tanh
                )
            return hb

        def l2(q, hb):
            ha = ps_ha.tile([TILE_P, NSUBQ, HID], F32, tag="ha")
            for s in range(NSUBQ):
                nc.tensor.transpose(
                    ha[:, s, :],
                    hb[:, s * TILE_P : (s + 1) * TILE_P],
                    ident[:HID, :HID],
                )
            mu2s = sm_p.tile([TILE_P, NSUBQ], F32, tag="mu2s")
            nc.vector.tensor_reduce(mu2s, ha, AX.X, ALU.add)
            scr2_t = y2_p.tile([TILE_P, NSUBQ, 200], BF16, tag="y2")
            scr2 = scr2_t[:, :, 0:HID]
            nc.scalar.activation(scr2, ha, AF.Square)
            sq2s = sm_p.tile([TILE_P, NSUBQ], F32, tag="sq2s")
            nc.vector.tensor_reduce(sq2s, scr2, AX.X, ALU.add)
            musq2 = sm_p.tile([TILE_P, NSUBQ], F32, tag="musq2")
            nc.vector.tensor_mul(musq2, mu2s, mu2s)
            v2 = sm_p.tile([TILE_P, NSUBQ], F32, tag="v2")
            nc.vector.scalar_tensor_tensor(
                v2, musq2, -1.0 / HID, sq2s, ALU.mult, ALU.add
            )
            r23 = sm_p.tile([TILE_P, NSUBQ, 1], F32, tag="r23")
            _newton_rsqrt(nc, nw_p, v2, r23[:, :, 0], TILE_P, NSUBQ,
                          1.0 / (HID - 1))
            mu23 = sm_p.tile([TILE_P, NSUBQ, 1], F32, tag="mu23")
            nc.vector.tensor_scalar(
                mu23[:, :, 0], mu2s, 1.0 / HID, None, ALU.mult
            )
            xn2 = xn2_p.tile([TILE_P, NSUBQ, HID], F32, tag="xn2")
            for s in range(NSUBQ):
                nc.vector.tensor_scalar(
                    xn2[:, s, :], ha[:, s, :], mu23[:, s, :],
                    r23[:, s, :], ALU.subtract, ALU.mult,
                )
            b2s = b2s_p.tile([TILE_P, PBLK], F32, tag="b2s")
            for cch in range(NCH):
                b2t = ps_b2t.tile([HID, 512], F32, tag="b2t")
                for j in range(4):
                    s = 4 * cch + j
                    nc.tensor.transpose(
                        b2t[:, j * TILE_P : (j + 1) * TILE_P], xn2[:, s, :], ident
                    )
                nc.vector.tensor_copy(
                    b2s[0:HID, cch * 512 : (cch + 1) * 512], b2t
                )
                nc.sync.dma_start(
                    out=b2s[HID:, cch * 512 : (cch + 1) * 512],
                    in_=b2s[0:HID, cch * 512 : (cch + 1) * 512],
                )
            b2 = b2_p.tile([TILE_P, H2C, PBLK], BF16, tag="b2")
            rb1 = rb_p.tile([TILE_P, PBLK], BF16, tag="rb1")
            rb2 = rb_p.tile([TILE_P, PBLK], BF16, tag="rb2")
            for ci, a in enumerate(ANC2):
                sqt = sq_p.tile([TILE_P, PBLK], F32, tag="sq2")
                nc.scalar.activation(
                    sqt, b2s, AF.Square, bias=cb2a[:, ci : ci + 1]
                )
                nc.scalar.activation(b2[:, a, :], sqt, AF.Exp, scale=float(-A))
            nc.scalar.activation(rb1, b2s, AF.Exp, scale=float(2 * A * D2))
            nc.scalar.activation(rb2, b2s, AF.Exp, scale=float(4 * A * D2))
            rpow = {1: rb1, 2: rb2}
            for c in [1, 2, 4, 5]:
                src, pw = PLAN2[c]
                eng = nc.gpsimd if ENG2[c] == 'gp' else nc.vector
                eng.tensor_tensor(b2[:, c, :], b2[:, src, :], rpow[pw], ALU.mult)
            for qq in range(NCH):
                ops = ps_o.tile([OUT_DIM, 512], F32, tag="ops")
                for idx, c in enumerate(MM2_ORDER):
                    nc.tensor.matmul(
                        ops,
                        w2[:, c, :],
                        b2[:, c, qq * 512 : (qq + 1) * 512],
                        start=(idx == 0),
                        stop=(idx == H2C - 1),
                    )
                osb = outs_p.tile([OUT_DIM, 512], F32, tag="osb")
                nc.scalar.copy(osb, ops)
                nc.sync.dma_start(
                    out=out_ext[:, q * PBLK + qq * 512 : q * PBLK + (qq + 1) * 512],
                    in_=osb,
                )

        xnts = {}
        hbs = {}
        for q in range(nq):
            y5t = y5_p.tile([TILE_P, NSUBQ, 500], BF16, tag="y5")
            stage_x(q, y5t)
            if q >= 1:
                hb = l1(q - 1, xnts.pop(q - 1))
                hbs[q - 1] = hb
            xnt = stage_rest(q, y5t)
            xnts[q] = xnt
            if q >= 1:
                l2(q - 1, hbs.pop(q - 1))
        hb = l1(nq - 1, xnts.pop(nq - 1))
        l2(nq - 1, hb)

    _split_sync_waits(nc)
    return nc


def _fit_projection(M, c16):
    A = RBF_A
    t = np.linspace(-6.0, 6.0, 4801)
    w = np.exp(-0.5 * t * t) + 1e-4
    cM = np.linspace(c16[0], c16[-1], M)
    G16 = np.exp(-A * (t[:, None] - np.asarray(c16)[None, :]) ** 2)
    GM = np.exp(-A * (t[:, None] - cM[None, :]) ** 2)
    Ws = np.sqrt(w)[:, None]
    P, *_ = np.linalg.lstsq(GM * Ws, G16 * Ws, rcond=None)
    return P


def _host_consts(c1_mu, c2_mu, centers):
    A = RBF_A
    c16 = np.asarray(centers, np.float64)
    cM1 = np.linspace(c16[0], c16[-1], M1)
    cM2 = np.linspace(c16[0], c16[-1], M2)
    P1 = _fit_projection(M1, c16)
    P2 = _fit_projection(M2, c16)
    c1t = np.einsum('mn,oin->oim', P1, c1_mu.astype(np.float64))
    c2t = np.einsum('mn,oin->oim', P2, c2_mu.astype(np.float64))
    # chain roots
    root1 = np.array([0] * 6 + [6] * 5)
    s1 = np.exp(A * (cM1 ** 2 - cM1[root1] ** 2))
    w1 = np.einsum('oim,m->imo', c1t, 1.0 / s1)  # [i, m, o]
    w1 = np.ascontiguousarray(w1.reshape(IN_DIM, M1 * HID)).astype(ml_dtypes.bfloat16)

    w2 = np.zeros((TILE_P, H2C, OUT_DIM), np.float64)
    cb2a = np.zeros((TILE_P, 2), np.float32)
    for p in range(TILE_P):
        u = p // HID
        i = p % HID
        for c in range(H2C):
            n = H2C * u + c
            rc = H2C * u + (0 if c < 3 else 3)
            s = np.exp(A * (cM2[n] ** 2 - cM2[rc] ** 2))
            w2[p, c, :] = c2t[:, i, n] / s
        cb2a[p, 0] = -cM2[H2C * u + 0]
        cb2a[p, 1] = -cM2[H2C * u + 3]
    w2 = np.ascontiguousarray(w2.reshape(TILE_P, H2C * OUT_DIM)).astype(ml_dtypes.bfloat16)
    cb1a = np.tile(
        np.array([[-cM1[ANC1[0]], -cM1[ANC1[1]]]], np.float32), (IN_DIM, 1)
    )
    ident = np.eye(TILE_P, dtype=np.float32)
    return w1, w2, cb1a, cb2a, ident


def kernel(x, c1_mu, c2_mu, centers):
    x = np.asarray(x)
    batch = x.shape[0]
    rows = batch // NCORES
    c1_mu = np.asarray(c1_mu, np.float32)
    c2_mu = np.asarray(c2_mu, np.float32)
    centers = np.asarray(centers, np.float32)

    w1, w2, cb1a, cb2a, ident = _host_consts(c1_mu, c2_mu, centers)
    nc = build_bass(centers, rows)

    # bf16 cast + k-major shuffle so pooling windows become contiguous blocks
    xb = x.astype(ml_dtypes.bfloat16)
    xs = np.ascontiguousarray(
        xb.reshape(batch, IN_DIM, POOLW).transpose(0, 2, 1).reshape(batch, SEQ)
    )

    in_maps = []
    for i in range(NCORES):
        in_maps.append(
            {
                "x": xs[i * rows : (i + 1) * rows],
                "w1": w1,
                "w2": w2,
                "cb1a": cb1a,
                "cb2a": cb2a,
                "ident": ident,
            }
        )
    trace = bool(int(os.environ.get("BASS_KERNEL_TRACE", "0")))
    if trace:
        sys.path.insert(0, "/root/.axon_site")
        _ensure_ntff_hook()
    res = run_bass_kernel_spmd(nc, in_maps, list(range(NCORES)), trace=trace)
    global LAST_RESULTS
    LAST_RESULTS = res
    out = np.empty((batch, OUT_DIM), np.float32)
    for i in range(NCORES):
        out[i * rows : (i + 1) * rows] = res.results[i]["out"].T
    return out


if __name__ == "__main__":
    xs = np.random.randn(BATCH, SEQ).astype(np.float32)
    c1 = (np.random.randn(HID, IN_DIM, NB) * 0.05).astype(np.float32)
    c2 = (np.random.randn(OUT_DIM, HID, NB) * 0.05).astype(np.float32)
    cen = np.linspace(-3, 3, NB).astype(np.float32)
    print(kernel(xs, c1, c2, cen)[:2])


# revision 24
# speedup vs baseline: 1.1716x; 1.1716x over previous
"""BayesianKAN ECG kernel v2 for Trainium2 (8 NeuronCores, data-parallel).

Per-core pipeline over quads of 2048 rows (4 quads/core):
  x (bf16, host k-major shuffled) --DMA--> [128,4,1000] tiles
  --gp add + DVE add-tree--> pooled[128,16,100] fp32
  --reduce/ttr stats + Newton rsqrt--> mean/rstd --ts apply--> xn
  --PE transpose--> xnt[100,2048] fp32 (via DMA PSUM->SBUF)
  L1 basis (M1=11 compressed RBF, anchors m=0,6):
    ACT: Square+Exp anchors, Exp ratios R1=e^{2AD x}, R2=e^{4AD x} (bf16)
    DVE/GP: R4=R2*R2 and 9 slice products G_m = G_src * R^pw
  mm1: 22 bf16 matmuls -> h[64,2048] -> Tanh -> hb
  L2: transpose roundtrip for norm stats, apply -> back-transpose
    -> replicate DMA -> b2s[128,2048] fp32
    basis (M2=12, 6 chunks x 2 halves, anchor chunks 0,3): ACT Square/Exp +
    4 chunk products; mm2: 12 bf16 matmuls -> out[5,2048] -> DMA out.
Numerics: basis compression via host least-squares projection (16->M RBFs),
ratio-recurrence constants folded into w1/w2 host-side. Expected rel err ~1.2e-2
(validated by sim_v2.py host simulation).
"""

import os
import sys
from contextlib import ExitStack

import numpy as np
import ml_dtypes

sys.path.insert(0, "/opt/trn_rl_repo")

import concourse.bass as bass
import concourse.tile as tile
from concourse import mybir
from concourse.bass_utils import run_bass_kernel_spmd

from concourse import bass2jax as _b2j

_orig_hook = _b2j.neuronx_cc_hook


def _dbg_hook(*a, **k):
    try:
        return _orig_hook(*a, **k)
    except BaseException:
        import traceback
        with open("/tmp/hook_err.txt", "w") as f:
            traceback.print_exc(file=f)
        raise


_b2j.neuronx_cc_hook = _dbg_hook

MAX_WAITS = 1


def _split_sync_waits(nc, limit=MAX_WAITS):
    """Walrus in this env rejects instructions with more than ~2 sync waits.

    Move excess waits onto same-engine NOPs inserted right before the
    offending instruction (in-order engines make this equivalent).
    """
    n_split = 0
    for block in nc.main_func.blocks:
        new_insts = []
        for inst in block.instructions:
            si = inst.sync_info
            waits = list(si.on_wait) if si is not None else []
            if len(waits) > limit:
                extra, keep = waits[:-limit], waits[-limit:]
                for k in range(0, len(extra), limit):
                    nop = mybir.InstNoOp(
                        name=f"{inst.name}-ws{k}",
                        sync_info=mybir.SyncInfo(
                            on_wait=extra[k : k + limit], on_update=[]
                        ),
                        bass_nofuse=True,
                        engine=inst.engine,
                    )
                    nc.register_instruction(nop, overwrite=True)
                    new_insts.append(nop)
                    n_split += 1
                si.on_wait = keep
                inst.sync_info = si
            new_insts.append(inst)
        block.instructions[:] = new_insts
    return n_split


BATCH = 65536
SEQ = 1000
IN_DIM = 100
POOLW = 10
HID = 64
OUT_DIM = 5
NB = 16
NCORES = 8
ROWS = BATCH // NCORES          # 8192 rows per core
TILE_P = 128
PBLK = 2048                     # rows per quad
NSUBQ = PBLK // TILE_P          # subtiles per quad
NBLK2 = NSUBQ // 2              # dma blocks per quad (2 subtiles each)
NCH = PBLK // 512               # 512-col chunks per quad

M1 = 11
M2 = 12
H2C = M2 // 2                   # 6 chunks
RBF_A = float(0.5 / 0.36)

# L1 slice plan: m -> (src_m, ratio_power)
ANC1 = (0, 6)
PLAN1 = {1: (0, 1), 2: (0, 2), 3: (1, 2), 4: (0, 4), 5: (1, 4),
         7: (6, 1), 8: (6, 2), 9: (7, 2), 10: (6, 4)}
# L2 chunk plan: c -> (src_c, ratio_power)
ANC2 = (0, 3)
PLAN2 = {1: (0, 1), 2: (0, 2), 4: (3, 1), 5: (3, 2)}
MM1_ORDER = [0, 6, 1, 7, 2, 8, 3, 9, 4, 5, 10]
MM2_ORDER = [0, 3, 1, 4, 2, 5]
# engine routing for slice/chunk products ('gp' or 'dve')
ENG1 = {m: 'dve' for m in PLAN1}
ENG2 = {c: 'dve' for c in PLAN2}

LAST_RESULTS = None

F32 = mybir.dt.float32
BF16 = mybir.dt.bfloat16
I32 = mybir.dt.int32
AF = mybir.ActivationFunctionType
ALU = mybir.AluOpType
AX = mybir.AxisListType
MAGIC = 0x5F3759DF


def _ensure_ntff_hook():
    """Synthesize antenv.axon_hooks (absent in this image) so trace=True works."""
    import types

    if "antenv.axon_hooks" in sys.modules:
        return
    mod = types.ModuleType("antenv.axon_hooks")
    mod._hook = None

    def set_axon_ntff_profile_hook(h):
        mod._hook = h

    def get_axon_ntff_profile_hook():
        return mod._hook

    mod.set_axon_ntff_profile_hook = set_axon_ntff_profile_hook
    mod.get_axon_ntff_profile_hook = get_axon_ntff_profile_hook
    sys.modules["antenv.axon_hooks"] = mod
    import antenv

    antenv.axon_hooks = mod
    try:
        from trn_agent_boot.trn_boot import _ntff_profile_via_ctypes

        hook = _ntff_profile_via_ctypes("/opt/axon/libaxon_pjrt.so")
        if hook is not None:
            set_axon_ntff_profile_hook(hook)
    except Exception as e:
        print("ntff hook setup failed:", e)


def _newton_rsqrt(nc, pool, var_ap, out_ap, n_par, n_free, scale):
    """out = rsqrt(var * scale) elementwise; Newton w/ bit-trick seed."""
    v = pool.tile([n_par, n_free], F32, tag="nw_v")
    y = pool.tile([n_par, n_free], F32, tag="nw_y")
    t = pool.tile([n_par, n_free], F32, tag="nw_t")
    nc.vector.tensor_scalar(v, var_ap, float(scale), None, ALU.mult)
    nc.vector.tensor_scalar(
        y.bitcast(I32), v.bitcast(I32), 1, None, ALU.logical_shift_right
    )
    nc.vector.tensor_scalar(
        y.bitcast(I32), y.bitcast(I32), -1, MAGIC, ALU.mult, ALU.add
    )
    for it in range(2):
        nc.vector.tensor_mul(t, y, y)
        nc.vector.tensor_mul(t, t, v)
        nc.vector.tensor_scalar(t, t, -0.5, 1.5, ALU.mult, ALU.add)
        nc.vector.tensor_mul(out_ap if it == 1 else y, y, t)


def build_bass(centers_np, rows=ROWS):
    nq = rows // PBLK
    assert rows % PBLK == 0
    nc = bass.Bass()

    c16 = np.asarray(centers_np, np.float64)
    cM1 = np.linspace(c16[0], c16[-1], M1)
    cM2 = np.linspace(c16[0], c16[-1], M2)
    D1 = float(cM1[1] - cM1[0])
    D2 = float(cM2[1] - cM2[0])
    A = RBF_A

    x_in = nc.declare_dram_parameter("x", [rows, SEQ], BF16, isOutput=False)
    w1_in = nc.declare_dram_parameter("w1", [IN_DIM, M1 * HID], BF16, isOutput=False)
    w2_in = nc.declare_dram_parameter("w2", [TILE_P, H2C * OUT_DIM], BF16, isOutput=False)
    cb1a_in = nc.declare_dram_parameter("cb1a", [IN_DIM, 2], F32, isOutput=False)
    cb2a_in = nc.declare_dram_parameter("cb2a", [TILE_P, 2], F32, isOutput=False)
    ident_in = nc.declare_dram_parameter("ident", [TILE_P, TILE_P], F32, isOutput=False)
    out_ext = nc.declare_dram_parameter("out", [OUT_DIM, rows], F32, isOutput=True)

    with ExitStack() as ctx:
        tc = ctx.enter_context(tile.TileContext(nc))
        singles = ctx.enter_context(tc.tile_pool(name="singles", bufs=1))
        xin_p = ctx.enter_context(tc.tile_pool(name="xin", bufs=2))
        y5_p = ctx.enter_context(tc.tile_pool(name="y5", bufs=1))
        y2_p = ctx.enter_context(tc.tile_pool(name="y2", bufs=1))
        pool_p = ctx.enter_context(tc.tile_pool(name="pooled", bufs=1))
        sm_p = ctx.enter_context(tc.tile_pool(name="sm", bufs=2))
        nw_p = ctx.enter_context(tc.tile_pool(name="newton", bufs=2))
        xn_p = ctx.enter_context(tc.tile_pool(name="xn", bufs=1))
        xnt_p = ctx.enter_context(tc.tile_pool(name="xnt", bufs=2))
        sq_p = ctx.enter_context(tc.tile_pool(name="sq", bufs=1))
        b1_p = ctx.enter_context(tc.tile_pool(name="b1", bufs=1))
        r1_p = ctx.enter_context(tc.tile_pool(name="r1", bufs=1))
        hb_p = ctx.enter_context(tc.tile_pool(name="hb", bufs=1))
        xn2_p = ctx.enter_context(tc.tile_pool(name="xn2", bufs=2))
        b2s_p = ctx.enter_context(tc.tile_pool(name="b2s", bufs=1))
        rb_p = ctx.enter_context(tc.tile_pool(name="rb", bufs=1))
        b2_p = ctx.enter_context(tc.tile_pool(name="b2", bufs=1))
        outs_p = ctx.enter_context(tc.tile_pool(name="outs", bufs=2))
        ps_xnt = ctx.enter_context(tc.tile_pool(name="ps_xnt", bufs=2, space="PSUM"))
        ps_h = ctx.enter_context(tc.tile_pool(name="ps_h", bufs=2, space="PSUM"))
        ps_ha = ctx.enter_context(tc.tile_pool(name="ps_ha", bufs=1, space="PSUM"))
        ps_b2t = ctx.enter_context(tc.tile_pool(name="ps_b2t", bufs=1, space="PSUM"))
        ps_o = ctx.enter_context(tc.tile_pool(name="ps_o", bufs=1, space="PSUM"))

        ident = singles.tile([TILE_P, TILE_P], F32)
        nc.sync.dma_start(out=ident, in_=ident_in[:, :])
        w1 = singles.tile([IN_DIM, M1, HID], BF16)
        nc.sync.dma_start(out=w1, in_=w1_in[:, :].rearrange("i (m o) -> i m o", m=M1))
        w2 = singles.tile([TILE_P, H2C, OUT_DIM], BF16)
        nc.sync.dma_start(out=w2, in_=w2_in[:, :].rearrange("p (c o) -> p c o", c=H2C))
        cb1a = singles.tile([IN_DIM, 2], F32)
        nc.sync.dma_start(out=cb1a, in_=cb1a_in[:, :])
        cb2a = singles.tile([TILE_P, 2], F32)
        nc.sync.dma_start(out=cb2a, in_=cb2a_in[:, :])

        # x view: [nq, blk, sub, p, f]
        x_t = x_in[:, :].rearrange(
            "(q b s p) f -> q b s p f", q=nq, b=NBLK2, s=2
        )

        def stage_x(q, y5):
            """Per-quad input DMA + first pooling add on gpsimd."""
            for b in range(NBLK2):
                xt = xin_p.tile([TILE_P, 2, SEQ], BF16, tag="x")
                nc.sync.dma_start(
                    out=xt, in_=x_t[q, b].rearrange("s p f -> p s f")
                )
                nc.gpsimd.tensor_tensor(
                    y5[:, 2 * b : 2 * b + 2, :],
                    xt[:, :, 0:500],
                    xt[:, :, 500:1000],
                    ALU.add,
                )

        def stage_rest(q, y5):
            """Pool tree + stats + normalize + transpose -> xnt SBUF."""
            y2 = y2_p.tile([TILE_P, NSUBQ, 200], BF16, tag="y2")
            nc.vector.tensor_tensor(
                y2, y5[:, :, 0:200], y5[:, :, 200:400], ALU.add
            )
            pooled = pool_p.tile([TILE_P, NSUBQ, IN_DIM], F32, tag="pooled")
            nc.vector.tensor_tensor(
                pooled, y2[:, :, 0:100], y2[:, :, 100:200], ALU.add
            )
            nc.vector.tensor_tensor(
                pooled, pooled, y5[:, :, 400:500], ALU.add
            )
            mu_s = sm_p.tile([TILE_P, NSUBQ], F32, tag="mu_s")
            nc.vector.tensor_reduce(mu_s, pooled, AX.X, ALU.add)
            scr_t = y2_p.tile([TILE_P, NSUBQ, 200], BF16, tag="y2")
            scr = scr_t[:, :, 0:IN_DIM]
            nc.scalar.activation(scr, pooled, AF.Square)
            sq_s = sm_p.tile([TILE_P, NSUBQ], F32, tag="sq_s")
            nc.vector.tensor_reduce(sq_s, scr, AX.X, ALU.add)
            musq = sm_p.tile([TILE_P, NSUBQ], F32, tag="musq")
            nc.vector.tensor_mul(musq, mu_s, mu_s)
            v = sm_p.tile([TILE_P, NSUBQ], F32, tag="v")
            nc.vector.scalar_tensor_tensor(
                v, musq, -1.0 / IN_DIM, sq_s, ALU.mult, ALU.add
            )
            r3 = sm_p.tile([TILE_P, NSUBQ, 1], F32, tag="r3")
            _newton_rsqrt(nc, nw_p, v, r3[:, :, 0], TILE_P, NSUBQ,
                          1.0 / (IN_DIM - 1))
            mur = sm_p.tile([TILE_P, NSUBQ, 1], F32, tag="mur")
            nc.vector.tensor_scalar(
                mur[:, :, 0], mu_s, -1.0 / IN_DIM, None, ALU.mult
            )
            nc.vector.tensor_mul(mur, mur, r3)
            xn_all = xn_p.tile([TILE_P, NSUBQ, IN_DIM], F32, tag="xn_all")
            for s in range(NSUBQ):
                nc.scalar.activation(
                    xn_all[:, s, :], pooled[:, s, :], AF.Identity,
                    bias=mur[:, s, :], scale=r3[:, s, :],
                )
            xnt = xnt_p.tile([IN_DIM, PBLK], F32, tag="xnt")
            for hq in range(NCH):
                xnt_ps = ps_xnt.tile([IN_DIM, 512], F32, tag="xnt_ps")
                for j in range(4):
                    s = 4 * hq + j
                    nc.tensor.transpose(
                        xnt_ps[:, j * TILE_P : (j + 1) * TILE_P],
                        xn_all[:, s, :], ident,
                    )
                nc.scalar.copy(xnt[:, hq * 512 : (hq + 1) * 512], xnt_ps)
            return xnt

        def l1(q, xnt):
            b1 = b1_p.tile([TILE_P, M1, PBLK], BF16, tag="b1")
            r1b = r1_p.tile([TILE_P, PBLK], BF16, tag="r1b")
            r2b = r1_p.tile([TILE_P, PBLK], BF16, tag="r2b")
            r4b = r1_p.tile([TILE_P, PBLK], BF16, tag="r4b")
            for ai, a in enumerate(ANC1):
                sqt = sq_p.tile([IN_DIM, PBLK], F32, tag="sq1")
                nc.scalar.activation(sqt, xnt, AF.Square, bias=cb1a[:, ai : ai + 1])
                nc.scalar.activation(
                    b1[0:IN_DIM, a, :], sqt, AF.Exp, scale=float(-A)
                )
            nc.scalar.activation(
                r1b[0:IN_DIM, :], xnt, AF.Exp, scale=float(2 * A * D1)
            )
            nc.scalar.activation(
                r2b[0:IN_DIM, :], xnt, AF.Exp, scale=float(4 * A * D1)
            )
            # pad rows hold zeros (memset once would cost more than it saves;
            # full-128 ops on them keep the DVE 2x packed mode engaged)
            nc.vector.tensor_mul(r4b, r2b, r2b)
            rpow = {1: r1b, 2: r2b, 4: r4b}
            # emit in dependency order
            for m in [1, 7, 2, 8, 3, 4, 9, 5, 10]:
                src, pw = PLAN1[m]
                eng = nc.gpsimd if ENG1[m] == 'gp' else nc.vector
                eng.tensor_tensor(b1[:, m, :], b1[:, src, :], rpow[pw], ALU.mult)
            hb = hb_p.tile([HID, PBLK], F32, tag="hb")
            for g in range(NCH):
                hps = ps_h.tile([HID, 512], F32, tag="hps")
                for idx, m in enumerate(MM1_ORDER):
                    nc.tensor.matmul(
                        hps,
                        w1[:, m, :],
                        b1[0:IN_DIM, m, g * 512 : (g + 1) * 512],
                        start=(idx == 0),
                        stop=(idx == M1 - 1),
                    )
                nc.scalar.activation(
                    hb[:, g * 512 : (g + 1) * 512], hps, AF.Tanh
                )
            return hb

        def l2(q, hb):
            ha = ps_ha.tile([TILE_P, NSUBQ, HID], F32, tag="ha")
            for s in range(NSUBQ):
                nc.tensor.transpose(
                    ha[:, s, :],
                    hb[:, s * TILE_P : (s + 1) * TILE_P],
                    ident[:HID, :HID],
                )
            mu2s = sm_p.tile([TILE_P, NSUBQ], F32, tag="mu2s")
            nc.vector.tensor_reduce(mu2s, ha, AX.X, ALU.add)
            scr2_t = y2_p.tile([TILE_P, NSUBQ, 200], BF16, tag="y2")
            scr2 = scr2_t[:, :, 0:HID]
            nc.scalar.activation(scr2, ha, AF.Square)
            sq2s = sm_p.tile([TILE_P, NSUBQ], F32, tag="sq2s")
            nc.vector.tensor_reduce(sq2s, scr2, AX.X, ALU.add)
            musq2 = sm_p.tile([TILE_P, NSUBQ], F32, tag="musq2")
            nc.vector.tensor_mul(musq2, mu2s, mu2s)
            v2 = sm_p.tile([TILE_P, NSUBQ], F32, tag="v2")
            nc.vector.scalar_tensor_tensor(
                v2, musq2, -1.0 / HID, sq2s, ALU.mult, ALU.add
            )
            r23 = sm_p.tile([TILE_P, NSUBQ, 1], F32, tag="r23")
            _newton_rsqrt(nc, nw_p, v2, r23[:, :, 0], TILE_P, NSUBQ,
                          1.0 / (HID - 1))
            mu23 = sm_p.tile([TILE_P, NSUBQ, 1], F32, tag="mu23")
            nc.vector.tensor_scalar(
                mu23[:, :, 0], mu2s, 1.0 / HID, None, ALU.mult
            )
            xn2 = xn2_p.tile([TILE_P, NSUBQ, HID], F32, tag="xn2")
            for s in range(NSUBQ):
                nc.vector.tensor_scalar(
                    xn2[:, s, :], ha[:, s, :], mu23[:, s, :],
                    r23[:, s, :], ALU.subtract, ALU.mult,
                )
            b2s = b2s_p.tile([TILE_P, PBLK], F32, tag="b2s")
            for cch in range(NCH):
                b2t = ps_b2t.tile([HID, 512], F32, tag="b2t")
                for j in range(4):
                    s = 4 * cch + j
                    nc.tensor.transpose(
                        b2t[:, j * TILE_P : (j + 1) * TILE_P], xn2[:, s, :], ident
                    )
                nc.vector.tensor_copy(
                    b2s[0:HID, cch * 512 : (cch + 1) * 512], b2t
                )
                nc.sync.dma_start(
                    out=b2s[HID:, cch * 512 : (cch + 1) * 512],
                    in_=b2s[0:HID, cch * 512 : (cch + 1) * 512],
                )
            b2 = b2_p.tile([TILE_P, H2C, PBLK], BF16, tag="b2")
            rb1 = rb_p.tile([TILE_P, PBLK], BF16, tag="rb1")
            rb2 = rb_p.tile([TILE_P, PBLK], BF16, tag="rb2")
            for ci, a in enumerate(ANC2):
                sqt = sq_p.tile([TILE_P, PBLK], F32, tag="sq2")
                nc.scalar.activation(
                    sqt, b2s, AF.Square, bias=cb2a[:, ci : ci + 1]
                )
                nc.scalar.activation(b2[:, a, :], sqt, AF.Exp, scale=float(-A))
            nc.scalar.activation(rb1, b2s, AF.Exp, scale=float(2 * A * D2))
            nc.scalar.activation(rb2, b2s, AF.Exp, scale=float(4 * A * D2))
            rpow = {1: rb1, 2: rb2}
            for c in [1, 2, 4, 5]:
                src, pw = PLAN2[c]
                eng = nc.gpsimd if ENG2[c] == 'gp' else nc.vector
                eng.tensor_tensor(b2[:, c, :], b2[:, src, :], rpow[pw], ALU.mult)
            for qq in range(NCH):
                ops = ps_o.tile([OUT_DIM, 512], F32, tag="ops")
                for idx, c in enumerate(MM2_ORDER):
                    nc.tensor.matmul(
                        ops,
                        w2[:, c, :],
                        b2[:, c, qq * 512 : (qq + 1) * 512],
                        start=(idx == 0),
                        stop=(idx == H2C - 1),
                    )
                osb = outs_p.tile([OUT_DIM, 512], F32, tag="osb")
                nc.scalar.copy(osb, ops)
                nc.sync.dma_start(
                    out=out_ext[:, q * PBLK + qq * 512 : q * PBLK + (qq + 1) * 512],
                    in_=osb,
                )

        xnts = {}
        hbs = {}
        for q in range(nq):
            y5t = y5_p.tile([TILE_P, NSUBQ, 500], BF16, tag="y5")
            stage_x(q, y5t)
            if q >= 1:
                hb = l1(q - 1, xnts.pop(q - 1))
                hbs[q - 1] = hb
            xnt = stage_rest(q, y5t)
            xnts[q] = xnt
            if q >= 1:
                l2(q - 1, hbs.pop(q - 1))
        hb = l1(nq - 1, xnts.pop(nq - 1))
        l2(nq - 1, hb)

    _split_sync_waits(nc)
    return nc


def _fit_projection(M, c16):
    A = RBF_A
    t = np.linspace(-6.0, 6.0, 4801)
    w = np.exp(-0.5 * t * t) + 1e-4
    cM = np.linspace(c16[0], c16[-1], M)
    G16 = np.exp(-A * (t[:, None] - np.asarray(c16)[None, :]) ** 2)
    GM = np.exp(-A * (t[:, None] - cM[None, :]) ** 2)
    Ws = np.sqrt(w)[:, None]
    P, *_ = np.linalg.lstsq(GM * Ws, G16 * Ws, rcond=None)
    return P


def _host_consts(c1_mu, c2_mu, centers):
    A = RBF_A
    c16 = np.asarray(centers, np.float64)
    cM1 = np.linspace(c16[0], c16[-1], M1)
    cM2 = np.linspace(c16[0], c16[-1], M2)
    P1 = _fit_projection(M1, c16)
    P2 = _fit_projection(M2, c16)
    c1t = np.einsum('mn,oin->oim', P1, c1_mu.astype(np.float64))
    c2t = np.einsum('mn,oin->oim', P2, c2_mu.astype(np.float64))
    # chain roots
    root1 = np.array([0] * 6 + [6] * 5)
    s1 = np.exp(A * (cM1 ** 2 - cM1[root1] ** 2))
    w1 = np.einsum('oim,m->imo', c1t, 1.0 / s1)  # [i, m, o]
    w1 = np.ascontiguousarray(w1.reshape(IN_DIM, M1 * HID)).astype(ml_dtypes.bfloat16)

    w2 = np.zeros((TILE_P, H2C, OUT_DIM), np.float64)
    cb2a = np.zeros((TILE_P, 2), np.float32)
    for p in range(TILE_P):
        u = p // HID
        i = p % HID
        for c in range(H2C):
            n = H2C * u + c
            rc = H2C * u + (0 if c < 3 else 3)
            s = np.exp(A * (cM2[n] ** 2 - cM2[rc] ** 2))
            w2[p, c, :] = c2t[:, i, n] / s
        cb2a[p, 0] = -cM2[H2C * u + 0]
        cb2a[p, 1] = -cM2[H2C * u + 3]
    w2 = np.ascontiguousarray(w2.reshape(TILE_P, H2C * OUT_DIM)).astype(ml_dtypes.bfloat16)
    cb1a = np.tile(
        np.array([[-cM1[ANC1[0]], -cM1[ANC1[1]]]], np.float32), (IN_DIM, 1)
    )
    ident = np.eye(TILE_P, dtype=np.float32)
    return w1, w2, cb1a, cb2a, ident


def kernel(x, c1_mu, c2_mu, centers):
    x = np.asarray(x)
    batch = x.shape[0]
    rows = batch // NCORES
    c1_mu = np.asarray(c1_mu, np.float32)
    c2_mu = np.asarray(c2_mu, np.float32)
    centers = np.asarray(centers, np.float32)

    w1, w2, cb1a, cb2a, ident = _host_consts(c1_mu, c2_mu, centers)
    nc = build_bass(centers, rows)

    # bf16 cast + k-major shuffle so pooling windows become contiguous blocks
    xb = x.astype(ml_dtypes.bfloat16)
    xs = np.ascontiguousarray(
        xb.reshape(batch, IN_DIM, POOLW).transpose(0, 2, 1).reshape(batch, SEQ)
    )

    in_maps = []
    for i in range(NCORES):
        in_maps.append(
            {
                "x": xs[i * rows : (i + 1) * rows],
                "w1": w1,
                "w2": w2,
                "cb1a": cb1a,
                "cb2a": cb2a,
                "ident": ident,
            }
        )
    trace = bool(int(os.environ.get("BASS_KERNEL_TRACE", "0")))
    if trace:
        sys.path.insert(0, "/root/.axon_site")
        _ensure_ntff_hook()
    res = run_bass_kernel_spmd(nc, in_maps, list(range(NCORES)), trace=trace)
    global LAST_RESULTS
    LAST_RESULTS = res
    out = np.empty((batch, OUT_DIM), np.float32)
    for i in range(NCORES):
        out[i * rows : (i + 1) * rows] = res.results[i]["out"].T
    return out


if __name__ == "__main__":
    xs = np.random.randn(BATCH, SEQ).astype(np.float32)
    c1 = (np.random.randn(HID, IN_DIM, NB) * 0.05).astype(np.float32)
    c2 = (np.random.randn(OUT_DIM, HID, NB) * 0.05).astype(np.float32)
    cen = np.linspace(-3, 3, NB).astype(np.float32)
    print(kernel(xs, c1, c2, cen)[:2])
